# revision 1
# baseline (speedup 1.0000x reference)
"""Trainium2 Bass kernel for the LIGHT temporal-shift motion block.

Data-parallel over clips: 8 cores x 1 clip (8 frames) each.
Per core: 1x1 conv (f32r matmul) -> global BN stats via AllReduce ->
BN+ReLU -> two 3x3 convs (f32r matmul, block-diagonal) -> temporal
shift-subtract -> output. Identity channels (64:256) stream through SBUF.
"""

import sys

sys.path.insert(0, "/opt/trn_rl_repo")
import numpy as np

import concourse.bacc as bacc
import concourse.mybir as mybir
import concourse.tile as tile
from concourse.bass_utils import run_bass_kernel_spmd

F32 = mybir.dt.float32
F32R = mybir.dt.float32r

N_CORES = 8
NF = 8  # frames per clip (n_segment)
C = 256
H = W = 56
S = H * W  # 3136
FOLD = 32
CZ = 2 * FOLD  # 64
PW = W + 2  # 58 padded row stride
PF = PW * (H + 2)  # 3364 padded frame size
NCHUNK = 7
CH = 8  # rows per chunk
CN = CH * W  # 448 matmul moving size
COUNT = 64 * S  # global BN count (all frames all clips)
BN_EPS = 1e-5

_CACHE = {}


def _build(n_cores=N_CORES, use_collective=True, compile_=True):
    key = (n_cores, use_collective)
    if key in _CACHE:
        return _CACHE[key]
    nc = bacc.Bacc("TRN2", target_bir_lowering=False, debug=False, num_devices=n_cores)
    x_d = nc.dram_tensor("x", [NF, C, S], F32R, kind="ExternalInput").ap()
    w1t_d = nc.dram_tensor("w1t", [C, CZ], F32R, kind="ExternalInput").ap()
    wtap_d = nc.dram_tensor("wtap", [CZ, 9 * CZ], F32R, kind="ExternalInput").ap()
    aux_d = nc.dram_tensor("aux", [CZ, 3], F32, kind="ExternalInput").ap()
    out_d = nc.dram_tensor("out", [NF, C, S], F32, kind="ExternalOutput").ap()

    AF = mybir.ActivationFunctionType
    AX = mybir.AxisListType
    ALU = mybir.AluOpType

    with tile.TileContext(nc) as tc:
        with (
            tc.tile_pool(name="persist", bufs=1) as pp,
            tc.tile_pool(name="psum", bufs=4, space="PSUM") as ps,
            tc.tile_pool(name="dram", bufs=1, space="DRAM") as dp,
        ):
            zpad = pp.tile([CZ, NF * PF + PW], F32R)
            w1t_t = pp.tile([128, 2 * CZ], F32R)
            wtap_t = pp.tile([CZ, 9 * CZ], F32R)
            aux_t = pp.tile([CZ, 3], F32)
            sum_t = pp.tile([CZ, 64], F32)
            sq_t = pp.tile([CZ, 64], F32)

            nc.vector.memset(zpad[:].bitcast(F32), 0.0)
            nc.vector.memset(sum_t[:], 0.0)
            nc.vector.memset(sq_t[:], 0.0)
            nc.sync.dma_start(w1t_t[:, 0:CZ], w1t_d[0:128, :])
            nc.sync.dma_start(w1t_t[:, CZ : 2 * CZ], w1t_d[128:256, :])
            nc.sync.dma_start(wtap_t[:], wtap_d[:])
            nc.sync.dma_start(aux_t[:], aux_d[:])

            def zap(f, c0, pn0, pn1, dy, dx, nrow=CH):
                # zpad interior AP: partitions [pn0,pn1), chunk rows
                # c0*8..+nrow shifted by (dy,dx); free dims (nrow, 56)
                base = f * PF + (c0 * CH + 1 + dy) * PW + 1 + dx
                v = zpad[pn0:pn1, base : base + nrow * PW]
                v = v.rearrange("p (a b) -> p a b", a=nrow, b=PW)[:, :, 0:W]
                return v

            # ---------- Phase A: load x, 1x1 conv, stats, identity out ----------
            with tc.tile_pool(name="xp", bufs=2) as xp:
                for f in range(NF):
                    x0 = xp.tile([128, S], F32R, tag="x0", name=f"x0_{f}")
                    x1 = xp.tile([128, S], F32R, tag="x1", name=f"x1_{f}")
                    nc.sync.dma_start(x0[:], x_d[f, 0:128, :])
                    nc.sync.dma_start(x1[:], x_d[f, 128:256, :])
                    nc.sync.dma_start(out_d[f, CZ:128, :], x0[CZ:128, :].bitcast(F32))
                    nc.sync.dma_start(out_d[f, 128:256, :], x1[:].bitcast(F32))
                    for c in range(NCHUNK):
                        zp = ps.tile([CZ, CN], F32, tag="zp", name=f"zp_{f}_{c}")
                        sl = slice(c * CN, (c + 1) * CN)
                        nc.tensor.matmul(
                            zp[:], w1t_t[:, 0:CZ], x0[:, sl], start=True, stop=False
                        )
                        nc.tensor.matmul(
                            zp[:], w1t_t[:, CZ : 2 * CZ], x1[:, sl],
                            start=False, stop=True,
                        )
                        dest = zap(f, c, 0, CZ, 0, 0)
                        src = zp[:].rearrange("p (a b) -> p a b", a=CH)
                        idx = f * NCHUNK + c
                        nc.scalar.activation(
                            dest, src, AF.Copy, accum_out=sum_t[:, idx : idx + 1]
                        )
                        nc.scalar.activation(
                            zp[:], zp[:], AF.Square,
                            accum_out=sq_t[:, idx : idx + 1],
                        )

            # ---------- Stats AllReduce + scale/shift ----------
            stats_t = pp.tile([CZ, 2], F32)
            nc.vector.tensor_reduce(
                stats_t[:, 0:1], sum_t[:, 0 : NF * NCHUNK], AX.X, ALU.add
            )
            nc.vector.tensor_reduce(
                stats_t[:, 1:2], sq_t[:, 0 : NF * NCHUNK], AX.X, ALU.add
            )
            gstats = pp.tile([CZ, 2], F32)
            if use_collective:
                cc_in = dp.tile([CZ, 2], F32)
                cc_out = dp.tile([CZ, 2], F32, addr_space="Shared")
                nc.sync.dma_start(cc_in[:], stats_t[:])
                nc.gpsimd.collective_compute(
                    "AllReduce",
                    ALU.add,
                    replica_groups=[list(range(n_cores))],
                    ins=[cc_in.opt()],
                    outs=[cc_out.opt()],
                )
                nc.sync.dma_start(gstats[:], cc_out[:])
            else:
                nc.vector.tensor_scalar_mul(gstats[:], stats_t[:], float(N_CORES))

            mean_t = pp.tile([CZ, 1], F32)
            var_t = pp.tile([CZ, 1], F32)
            std_t = pp.tile([CZ, 1], F32)
            rstd_t = pp.tile([CZ, 1], F32)
            scale_t = pp.tile([CZ, 1], F32)
            shift_t = pp.tile([CZ, 1], F32)
            tmp_t = pp.tile([CZ, 1], F32)
            inv = 1.0 / COUNT
            nc.vector.tensor_scalar_mul(mean_t[:], gstats[:, 0:1], inv)
            nc.vector.tensor_scalar_mul(var_t[:], gstats[:, 1:2], inv)
            nc.vector.tensor_mul(tmp_t[:], mean_t[:], mean_t[:])
            nc.vector.tensor_sub(var_t[:], var_t[:], tmp_t[:])
            nc.vector.tensor_scalar_add(var_t[:], var_t[:], BN_EPS)
            nc.scalar.sqrt(std_t[:], var_t[:])
            nc.vector.reciprocal(rstd_t[:], std_t[:])
            nc.vector.tensor_mul(scale_t[:], aux_t[:, 1:2], rstd_t[:])
            nc.vector.tensor_mul(tmp_t[:], mean_t[:], scale_t[:])
            nc.vector.tensor_sub(shift_t[:], aux_t[:, 2:3], tmp_t[:])

            # ---------- Phase C: BN+ReLU, 3x3 convs, shift-subtract ----------
            for f in range(NF):
                v = zap(f, 0, 0, CZ, 0, 0, nrow=H)
                nc.scalar.activation(
                    v, v.bitcast(F32), AF.Relu, bias=shift_t[:], scale=scale_t[:]
                )

            with tc.tile_pool(name="stg", bufs=4) as sp:
                stg = {}
                stg[0] = sp.tile([CZ, S], F32, tag="stg", name="stg_0")
                nc.vector.memset(stg[0][FOLD:CZ, :], 0.0)
                for f in range(NF):
                    if f < NF - 1:
                        stg[f + 1] = sp.tile(
                            [CZ, S], F32, tag="stg", name=f"stg_{f + 1}"
                        )
                        if f == NF - 2:
                            nc.vector.memset(stg[NF - 1][0:FOLD, :], 0.0)
                    for c in range(NCHUNK):
                        cp = ps.tile([CZ, CN], F32, tag="cp", name=f"cp_{f}_{c}")
                        t = 0
                        for dy in (-1, 0, 1):
                            for dx in (-1, 0, 1):
                                nc.tensor.matmul(
                                    cp[:],
                                    wtap_t[:, t * CZ : (t + 1) * CZ],
                                    zap(f, c, 0, CZ, dy, dx),
                                    start=(t == 0),
                                    stop=(t == 8),
                                )
                                t += 1
                        cpr = cp[:].rearrange("p (a b) -> p a b", a=CH)
                        sl = slice(c * CN, (c + 1) * CN)
                        if f >= 1:
                            # out_a[f-1] = (nxt[f] + b_next) - za[f-1]
                            dsta = stg[f - 1][0:FOLD, sl].rearrange(
                                "p (a b) -> p a b", a=CH
                            )
                            nc.vector.scalar_tensor_tensor(
                                dsta,
                                cpr[0:FOLD],
                                aux_t[0:FOLD, 0:1],
                                zap(f - 1, c, 0, FOLD, 0, 0).bitcast(F32),
                                op0=ALU.add,
                                op1=ALU.subtract,
                            )
                        if f <= NF - 2:
                            # out_b[f+1] = (neglst[f] + (-b_last)) + zb[f+1]
                            dstb = stg[f + 1][FOLD:CZ, sl].rearrange(
                                "p (a b) -> p a b", a=CH
                            )
                            nc.vector.scalar_tensor_tensor(
                                dstb,
                                cpr[FOLD:CZ],
                                aux_t[FOLD:CZ, 0:1],
                                zap(f + 1, c, FOLD, CZ, 0, 0).bitcast(F32),
                                op0=ALU.add,
                                op1=ALU.add,
                            )
                    if f >= 1:
                        nc.sync.dma_start(out_d[f - 1, 0:CZ, :], stg[f - 1][:])
                nc.sync.dma_start(out_d[NF - 1, 0:CZ, :], stg[NF - 1][:])

    if compile_:
        nc.compile()
    _CACHE[key] = nc
    return nc


def _prep_weights(w1, b1, w_next, b_next, w_last, b_last, gamma, beta):
    w1t = np.ascontiguousarray(w1.reshape(CZ, C).T).astype(np.float32)
    wtap = np.zeros((CZ, 9 * CZ), np.float32)
    for t in range(9):
        dy, dx = t // 3, t % 3
        blk = np.zeros((CZ, CZ), np.float32)
        blk[0:FOLD, 0:FOLD] = w_next[:, :, dy, dx].T
        blk[FOLD:CZ, FOLD:CZ] = -w_last[:, :, dy, dx].T
        wtap[:, t * CZ : (t + 1) * CZ] = blk
    aux = np.zeros((CZ, 3), np.float32)
    aux[0:FOLD, 0] = b_next
    aux[FOLD:CZ, 0] = -b_last
    aux[:, 1] = gamma
    aux[:, 2] = beta
    return w1t, wtap, aux


def kernel(**inputs):
    x = np.asarray(inputs["x"], dtype=np.float32)
    w1t, wtap, aux = _prep_weights(
        np.asarray(inputs["w1"], np.float32),
        np.asarray(inputs["b1"], np.float32),
        np.asarray(inputs["w_next"], np.float32),
        np.asarray(inputs["b_next"], np.float32),
        np.asarray(inputs["w_last"], np.float32),
        np.asarray(inputs["b_last"], np.float32),
        np.asarray(inputs["gamma"], np.float32),
        np.asarray(inputs["beta"], np.float32),
    )
    nc = _build()
    xr = x.reshape(N_CORES, NF, C, S)
    in_maps = [
        {"x": np.ascontiguousarray(xr[c]), "w1t": w1t, "wtap": wtap, "aux": aux}
        for c in range(N_CORES)
    ]
    res = run_bass_kernel_spmd(nc, in_maps, core_ids=list(range(N_CORES)))
    out = np.stack([res.results[c]["out"] for c in range(N_CORES)], axis=0)
    return out.reshape(N_CORES * NF, C, H, W)

